# revision 1
# baseline (speedup 1.0000x reference)
"""Trainium2 Bass kernel v2 for upsample_conv_2d — fp16 datapath, big DMAs.

Same phase-decomposed math as the baseline (conv_transpose stride-2 3x3 +
4x4 FIR composed into four 3x3 convs on the 64x64 grid), but:
  - all SBUF tensors fp16 (PSUM accumulation stays fp32): matmul runs at
    full PE rate, LDWEIGHTS gets fast-weight-load, DMA bytes halve
  - output staged per (half, cob) as one [128, 64, 128] tile and shipped
    in 4 DMAs of 2MB with 16KB-contiguous-per-partition descriptors
    (the baseline's 16 x 1MB strided DMAs were the 61ms bottleneck)
  - DRAM output is fp16; the host upcasts to fp32 (rel-err budget 2e-2,
    fp16 rounding adds ~1e-3)
"""

import json

import numpy as np

import concourse.bass as bass
import concourse.mybir as mybir
import concourse.tile as tile
from concourse.bass_utils import run_bass_kernel_spmd

# ---------------------------------------------------------------------------
# BIR post-pass: this walrus build rejects instructions carrying more than one
# sem wait (e.g. Tile's kernel-tail Drain gets 3). Hoist extras into
# standalone EventSemaphore instructions right before the owner.
# ---------------------------------------------------------------------------
_MAX_WAITS = 1


def _split_waits(j: dict) -> dict:
    for fn in j.get("functions", []):
        for blk in fn.get("blocks", []):
            insts = blk.get("instructions")
            if not insts:
                continue
            out = []
            for inst in insts:
                si = inst.get("sync_info") or {}
                waits = si.get("on_wait") or []
                if len(waits) > _MAX_WAITS:
                    for k, w in enumerate(waits[_MAX_WAITS:]):
                        out.append(
                            {
                                "debug": inst.get("debug", 0),
                                "engine": inst["engine"],
                                "ins": [],
                                "name": f"{inst['name']}-wsplit{k}",
                                "opcode": "EventSemaphore",
                                "outs": [],
                                "sync_info": {"on_update": [], "on_wait": [w]},
                            }
                        )
                    si["on_wait"] = waits[:_MAX_WAITS]
                out.append(inst)
            blk["instructions"] = out
    return j


_orig_to_json_bytes = bass.Bass.to_json_bytes


def _patched_to_json_bytes(self):
    return json.dumps(_split_waits(json.loads(_orig_to_json_bytes(self)))).encode()


bass.Bass.to_json_bytes = _patched_to_json_bytes

# ---------------------------------------------------------------------------
# Problem constants (hardcoded; kernel.py must be self-contained)
# ---------------------------------------------------------------------------
N, C, H, W = 8, 256, 64, 64
OH, OW = 2 * H, 2 * W
N_CORES = 8
F32 = mybir.dt.float32
F16 = mybir.dt.float16

# tap order shared by host weight layout and device loop
_TAPS = [(e, f, cib) for e in (-1, 0, 1) for f in (-1, 0, 1) for cib in (0, 1)]


def _phase_weight_matrix(w: np.ndarray) -> np.ndarray:
    """[256,256,3,3] conv_transpose weight -> [128, 144*128] fp16 lhsT matrix.

    Column block index = ((cib*4 + ph)*9 + tap)*2 + cob, each 128 co wide;
    row = ci within ci-block. ph = pa*2+pb, tap = (e+1)*3+(f+1).
    """
    k1 = np.array([1.0, 3.0, 3.0, 1.0], dtype=np.float64)
    fir = np.outer(k1, k1)
    fir = fir / fir.sum() * 4.0  # gain = factor^2
    wd = w.astype(np.float64)
    # G[d1+3, d2+3] = sum_{p-u=d1, q-v=d2} w[p,q] fir[u,v]
    G = np.zeros((C, C, 6, 6), dtype=np.float64)
    for p in range(3):
        for q in range(3):
            for u in range(4):
                for v in range(4):
                    G[:, :, p - u + 3, q - v + 3] += wd[:, :, p, q] * fir[u, v]
    Wmat = np.zeros((128, 2 * 4 * 9 * 2, 128), dtype=np.float16)
    for cib in range(2):
        for pa in range(2):
            for pb in range(2):
                ph = pa * 2 + pb
                for e in (-1, 0, 1):
                    for f in (-1, 0, 1):
                        tap = (e + 1) * 3 + (f + 1)
                        # K[o, c] = G[o, c, 2e-pa+3, 2f-pb+3]
                        Kof = G[:, :, 2 * e - pa + 3, 2 * f - pb + 3]
                        for cob in range(2):
                            cidx = ((cib * 4 + ph) * 9 + tap) * 2 + cob
                            blk = Kof[
                                cob * 128 : (cob + 1) * 128,
                                cib * 128 : (cib + 1) * 128,
                            ]  # [co, ci]
                            Wmat[:, cidx, :] = blk.T.astype(np.float16)
    return Wmat.reshape(128, -1)


def _widx(cib: int, ph: int, tap: int, cob: int) -> int:
    return ((cib * 4 + ph) * 9 + tap) * 2 + cob


def build_nc(reps: int = 1, loop: bool = False) -> bass.Bass:
    nc = bass.Bass("TRN2", target_bir_lowering=False, debug=False)
    x_d = nc.dram_tensor("x", [C, H + 2, W + 2], F16, kind="ExternalInput").ap()
    w_d = nc.dram_tensor("w", [128, 144 * 128], F16, kind="ExternalInput").ap()
    b_d = nc.dram_tensor("bias", [2, 128], F32, kind="ExternalInput").ap()
    out_d = nc.dram_tensor("out", [C, OH, OW], F16, kind="ExternalOutput").ap()

    xb = x_d.rearrange("(b p) h w -> b p h w", p=128)

    with tile.TileContext(nc) as tc:
        with (
            tc.tile_pool(name="weights", bufs=1) as wpool,
            tc.tile_pool(name="xin", bufs=1) as xpool,
            tc.tile_pool(name="psum", bufs=8, space="PSUM") as ppool,
            tc.tile_pool(name="outs", bufs=4) as opool,
        ):
            wt = wpool.tile([128, 144, 128], F16)
            nc.sync.dma_start(wt[:], w_d.rearrange("p (a b) -> p a b", b=128))
            bt = wpool.tile([128, 2], F32)
            nc.sync.dma_start(bt[:], b_d.rearrange("b p -> p b"))

            # x arrives zero-padded to 66x66 from the host
            xpad = [
                xpool.tile([128, H + 2, W + 2], F16, tag=f"xp{i}", name=f"xp{i}")
                for i in range(2)
            ]
            for cib in range(2):
                nc.sync.dma_start(xpad[cib][:], xb[cib])

            def body():
                for half in range(2):
                    for cob in range(2):
                        T = opool.tile([128, 64, 128], F16, tag="st", name="st")
                        Tv = T[:].rearrange(
                            "p (k r a) (s b) -> p k r a s b", k=4, a=2, b=2
                        )
                        for pa in range(2):
                            for pb in range(2):
                                ph = pa * 2 + pb
                                psums = [
                                    ppool.tile([128, 8, 64], F32, tag="ps", name="ps")
                                    for _ in range(4)
                                ]
                                for it, (e, f, cib) in enumerate(_TAPS):
                                    tap = (e + 1) * 3 + (f + 1)
                                    lhsT = wt[:, _widx(cib, ph, tap, cob), :]
                                    for k in range(4):
                                        hb = half * 4 + k
                                        r0 = hb * 8 + 1 + e
                                        rhs = xpad[cib][
                                            :, r0 : r0 + 8, 1 + f : 65 + f
                                        ]
                                        nc.tensor.matmul(
                                            psums[k][:],
                                            lhsT,
                                            rhs,
                                            start=(it == 0),
                                            stop=(it == len(_TAPS) - 1),
                                        )
                                for k in range(4):
                                    nc.scalar.activation(
                                        Tv[:, k, :, pa, :, pb],
                                        psums[k][:],
                                        mybir.ActivationFunctionType.Identity,
                                        bias=bt[:, cob : cob + 1],
                                        scale=1.0,
                                    )
                        dst = out_d[
                            cob * 128 : (cob + 1) * 128,
                            half * 64 : half * 64 + 64,
                            :,
                        ].rearrange("c h w -> c (h w)")
                        nc.sync.dma_start(dst, T[:].rearrange("p h w -> p (h w)"))

            if loop:
                with tc.For_i(0, reps):
                    body()
            else:
                for _rep in range(reps):
                    body()
    return nc


_CACHED_NC = {}


def _get_nc(reps: int = 1, loop: bool = False) -> bass.Bass:
    key = (reps, loop)
    if key not in _CACHED_NC:
        _CACHED_NC[key] = build_nc(reps, loop)
    return _CACHED_NC[key]


def _run(x, weight, bias, reps: int = 1, loop: bool = False):
    Wmat = _phase_weight_matrix(np.asarray(weight, dtype=np.float32))
    b2 = np.ascontiguousarray(np.asarray(bias, dtype=np.float32).reshape(2, 128))
    xs = np.pad(
        np.asarray(x, dtype=np.float32), ((0, 0), (0, 0), (1, 1), (1, 1))
    ).astype(np.float16)
    nc = _get_nc(reps, loop)
    in_maps = [{"x": xs[i], "w": Wmat, "bias": b2} for i in range(N_CORES)]
    res = run_bass_kernel_spmd(nc, in_maps, list(range(N_CORES)))
    return np.stack(
        [res.results[i]["out"].astype(np.float32) for i in range(N_CORES)]
    )


def kernel(x, weight, bias):
    return _run(x, weight, bias, reps=1)



# revision 3
# speedup vs baseline: 1.9185x; 1.9185x over previous
"""Trainium2 Bass kernel v3 for upsample_conv_2d — polyphase conv + DVE FIR.

The baseline folded the 4x4 FIR into the conv weights, turning the op into
four phase-specific 3x3 convs (36 dense 256x256 taps, ~590K PE columns).
This version computes the mathematically minimal dense work instead:

  1. Stage A (PE): the stride-2 3x3 conv_transpose as 9 polyphase taps
     (~150K PE columns, 3.9x less tensor work). PSUM accumulates fp32;
     the Scalar engine evicts each 4-bank PSUM group to an SBUF fp16
     buffer z[131,131] with the two output phases interleaved (strided
     writes) and a zero ring for the FIR padding.
  2. Stage B (DVE): the separable FIR [1,3,3,1]/2 per dim decomposed as
     three 2-tap box filters per dim ([1,3,3,1] = [1,1]*[1,1]*[1,1]),
     run as scalar_tensor_tensor ops (fp16, SBUF, packed => 4x DVE perf
     mode). The final pass adds the conv bias via its per-partition
     scalar operand. The 1/16 FIR normalization is folded into the
     host-side x and w scaling (0.25 each).

Output is staged packed [128, 128*128] fp16 and shipped in 2 DMAs of
2 MB per co-block (16 KB contiguous per partition).
"""

import json

import numpy as np

import concourse.bass as bass
import concourse.mybir as mybir
import concourse.tile as tile
from concourse.bass_utils import run_bass_kernel_spmd

# ---------------------------------------------------------------------------
# BIR post-pass: this walrus build rejects instructions carrying more than one
# sem wait (e.g. Tile's kernel-tail Drain gets 3). Hoist extras into
# standalone EventSemaphore instructions right before the owner.
# ---------------------------------------------------------------------------
_MAX_WAITS = 1


def _split_waits(j: dict) -> dict:
    for fn in j.get("functions", []):
        for blk in fn.get("blocks", []):
            insts = blk.get("instructions")
            if not insts:
                continue
            out = []
            for inst in insts:
                si = inst.get("sync_info") or {}
                waits = si.get("on_wait") or []
                if len(waits) > _MAX_WAITS:
                    for k, w in enumerate(waits[_MAX_WAITS:]):
                        out.append(
                            {
                                "debug": inst.get("debug", 0),
                                "engine": inst["engine"],
                                "ins": [],
                                "name": f"{inst['name']}-wsplit{k}",
                                "opcode": "EventSemaphore",
                                "outs": [],
                                "sync_info": {"on_update": [], "on_wait": [w]},
                            }
                        )
                    si["on_wait"] = waits[:_MAX_WAITS]
                out.append(inst)
            blk["instructions"] = out
    return j


_orig_to_json_bytes = bass.Bass.to_json_bytes


def _patched_to_json_bytes(self):
    return json.dumps(_split_waits(json.loads(_orig_to_json_bytes(self)))).encode()


bass.Bass.to_json_bytes = _patched_to_json_bytes

# ---------------------------------------------------------------------------
# Problem constants (hardcoded; kernel.py must be self-contained)
# ---------------------------------------------------------------------------
N, C, H, W = 8, 256, 64, 64
OH, OW = 2 * H, 2 * W
N_CORES = 8
F32 = mybir.dt.float32
F16 = mybir.dt.float16

# Polyphase taps of the stride-2 conv_transpose with full padding:
#   y[2I+pv, 2J+ph] = sum_taps w[p, q] * xpad[I + dr, J + dc]
# where xpad has a 1-px zero halo (xpad[1+i, 1+j] = x[i, j]).
# Entries: (pv, ph, dr, dc, p, q)
TAPS = [
    (0, 0, 0, 0, 0, 0),
    (0, 0, 0, 1, 0, 2),
    (0, 0, 1, 0, 2, 0),
    (0, 0, 1, 1, 2, 2),
    (0, 1, 0, 1, 0, 1),
    (0, 1, 1, 1, 2, 1),
    (1, 0, 1, 0, 1, 0),
    (1, 0, 1, 1, 1, 2),
    (1, 1, 1, 1, 1, 1),
]

# y row/col counts per phase: even phase has 65 outputs, odd has 64.
PHASE_DIMS = {0: 65, 1: 64}


def _tap_weight_matrix(w: np.ndarray) -> np.ndarray:
    """[256,256,3,3] conv_transpose weight -> [128, 36*128] fp16 lhsT matrix.

    Column block index = (tap*2 + cib)*2 + cob; row = ci within ci-block.
    Scaled by 0.25 (with x also scaled 0.25 => 1/16 FIR normalization).
    """
    ws = w.astype(np.float64) * 0.25
    Wmat = np.zeros((128, 36, 128), dtype=np.float16)
    for t, (_pv, _ph, _dr, _dc, p, q) in enumerate(TAPS):
        for cib in range(2):
            for cob in range(2):
                blk = ws[
                    cob * 128 : (cob + 1) * 128,
                    cib * 128 : (cib + 1) * 128,
                    p,
                    q,
                ]  # [co, ci]
                Wmat[:, (t * 2 + cib) * 2 + cob, :] = blk.T.astype(np.float16)
    return Wmat.reshape(128, -1)


def build_nc(reps: int = 1, loop: bool = False) -> bass.Bass:
    nc = bass.Bass("TRN2", target_bir_lowering=False, debug=False)
    x_d = nc.dram_tensor("x", [C, H + 2, W + 2], F16, kind="ExternalInput").ap()
    w_d = nc.dram_tensor("w", [128, 36 * 128], F16, kind="ExternalInput").ap()
    b_d = nc.dram_tensor("bias", [2, 128], F32, kind="ExternalInput").ap()
    out_d = nc.dram_tensor("out", [C, OH, OW], F16, kind="ExternalOutput").ap()

    xb = x_d.rearrange("(b p) h w -> b p h w", p=128)

    mult = mybir.AluOpType.mult
    add = mybir.AluOpType.add
    COPY = mybir.ActivationFunctionType.Copy

    with tile.TileContext(nc) as tc:
        with (
            tc.tile_pool(name="const", bufs=1) as cpool,
            tc.tile_pool(name="zbuf", bufs=1) as zpool,
            tc.tile_pool(name="psum", bufs=2, space="PSUM") as ppool,
        ):
            wt = cpool.tile([128, 36, 128], F16)
            nc.sync.dma_start(wt[:], w_d.rearrange("p (a b) -> p a b", b=128))
            bt = cpool.tile([128, 2], F32)
            nc.sync.dma_start(bt[:], b_d.rearrange("b p -> p b"))

            # x arrives zero-padded to 66x66 (and pre-scaled 0.25) from host
            xpad = [
                cpool.tile([128, 66, 66], F16, tag=f"xp{i}", name=f"xp{i}")
                for i in range(2)
            ]
            for cib in range(2):
                nc.sync.dma_start(xpad[cib][:], xb[cib])

            # z: conv_transpose output (y/16) with 1-px zero ring, phases
            # interleaved. One per co-block so eviction of block 1 overlaps
            # the FIR of block 0. tA/tB: FIR ping-pong scratch.
            zb = [
                zpool.tile([128, 131, 131], F16, tag=f"z{i}", name=f"z{i}")
                for i in range(2)
            ]
            tA = zpool.tile([128, 131, 131], F16, tag="tA", name="tA")
            tB = zpool.tile([128, 131, 131], F16, tag="tB", name="tB")
            # one-time ring init (interior is rewritten every rep)
            for i in range(2):
                nc.vector.memset(zb[i][:], 0.0)

            def body():
                for cob in range(2):
                    z = zb[cob]
                    zv = z[:, 1:131, 1:131].rearrange(
                        "p (i a) (j b) -> p i a j b", a=2, b=2
                    )
                    # ---- Stage A: polyphase matmuls + PSUM eviction ----
                    for pv in (0, 1):
                        for ph in (0, 1):
                            accums = [
                                (dr, dc, cib, (t * 2 + cib) * 2 + cob)
                                for t, (tpv, tph, dr, dc, _p, _q) in enumerate(TAPS)
                                if tpv == pv and tph == ph
                                for cib in range(2)
                            ]
                            rows = PHASE_DIMS[pv]
                            cols = PHASE_DIMS[ph]
                            R = 7 if cols == 65 else 8
                            na = len(accums)
                            i0 = 0
                            while i0 < rows:
                                g_rows = min(4 * R, rows - i0)
                                chunks = []
                                r0 = i0
                                while r0 < i0 + g_rows:
                                    cr = min(R, i0 + g_rows - r0)
                                    chunks.append((r0, cr))
                                    r0 += cr
                                P = ppool.tile([128, 4, 512], F32, tag="mm", name="mm")
                                for ai, (dr, dc, cib, widx) in enumerate(accums):
                                    lhsT = wt[:, widx, :]
                                    for ci_, (cr0, crn) in enumerate(chunks):
                                        rhs = xpad[cib][
                                            :, cr0 + dr : cr0 + dr + crn, dc : dc + cols
                                        ]
                                        outp = P[:, ci_, 0 : crn * cols].rearrange(
                                            "p (r w) -> p r w", w=cols
                                        )
                                        nc.tensor.matmul(
                                            outp,
                                            lhsT,
                                            rhs,
                                            start=(ai == 0),
                                            stop=(ai == na - 1),
                                        )
                                # evict: leading full-R chunks in one op
                                nfull = 0
                                for _cr0, crn in chunks:
                                    if crn == R:
                                        nfull += 1
                                    else:
                                        break
                                if nfull:
                                    src = P[:, 0:nfull, 0 : R * cols].rearrange(
                                        "p c (r w) -> p c r w", w=cols
                                    )
                                    dst = zv[
                                        :, i0 : i0 + nfull * R, pv, 0:cols, ph
                                    ].rearrange("p (c r) j -> p c r j", r=R)
                                    nc.scalar.activation(dst, src, COPY)
                                for ci_ in range(nfull, len(chunks)):
                                    cr0, crn = chunks[ci_]
                                    src = P[:, ci_, 0 : crn * cols].rearrange(
                                        "p (r w) -> p r w", w=cols
                                    )
                                    dst = zv[:, cr0 : cr0 + crn, pv, 0:cols, ph]
                                    nc.scalar.activation(dst, src, COPY)
                                i0 += g_rows

                    # ---- Stage B: separable FIR as 6 box passes on DVE ----
                    stt = nc.vector.scalar_tensor_tensor
                    stt(
                        tA[:, 0:130, 0:131],
                        z[:, 0:130, 0:131],
                        1.0,
                        z[:, 1:131, 0:131],
                        mult,
                        add,
                    )
                    stt(
                        tB[:, 0:129, 0:131],
                        tA[:, 0:129, 0:131],
                        1.0,
                        tA[:, 1:130, 0:131],
                        mult,
                        add,
                    )
                    stt(
                        tA[:, 0:128, 0:131],
                        tB[:, 0:128, 0:131],
                        1.0,
                        tB[:, 1:129, 0:131],
                        mult,
                        add,
                    )
                    stt(
                        tB[:, 0:128, 0:130],
                        tA[:, 0:128, 0:130],
                        1.0,
                        tA[:, 0:128, 1:131],
                        mult,
                        add,
                    )
                    stt(
                        tA[:, 0:128, 0:129],
                        tB[:, 0:128, 0:129],
                        1.0,
                        tB[:, 0:128, 1:130],
                        mult,
                        add,
                    )
                    # final pass: += bias via the per-partition scalar operand,
                    # packed into tB viewed flat for contiguous DMA
                    tBf = tB[:].rearrange("p a b -> p (a b)")
                    ov = tBf[:, 0:16384].rearrange("p (h w) -> p h w", w=128)
                    stt(
                        ov,
                        tA[:, 0:128, 0:128],
                        bt[:, cob : cob + 1],
                        tA[:, 0:128, 1:129],
                        add,
                        add,
                    )
                    for half in range(2):
                        dst = out_d[
                            cob * 128 : (cob + 1) * 128,
                            half * 64 : (half + 1) * 64,
                            :,
                        ].rearrange("c h w -> c (h w)")
                        nc.sync.dma_start(
                            dst, tBf[:, half * 8192 : (half + 1) * 8192]
                        )

            if loop:
                with tc.For_i(0, reps):
                    body()
            else:
                for _rep in range(reps):
                    body()
    return nc


_CACHED_NC = {}


def _get_nc(reps: int = 1, loop: bool = False) -> bass.Bass:
    key = (reps, loop)
    if key not in _CACHED_NC:
        _CACHED_NC[key] = build_nc(reps, loop)
    return _CACHED_NC[key]


def _run(x, weight, bias, reps: int = 1, loop: bool = False):
    Wmat = _tap_weight_matrix(np.asarray(weight, dtype=np.float32))
    b2 = np.ascontiguousarray(np.asarray(bias, dtype=np.float32).reshape(2, 128))
    xs = (
        np.pad(np.asarray(x, dtype=np.float32), ((0, 0), (0, 0), (1, 1), (1, 1)))
        * 0.25
    ).astype(np.float16)
    nc = _get_nc(reps, loop)
    in_maps = [{"x": xs[i], "w": Wmat, "bias": b2} for i in range(N_CORES)]
    res = run_bass_kernel_spmd(nc, in_maps, list(range(N_CORES)))
    return np.stack(
        [res.results[i]["out"].astype(np.float32) for i in range(N_CORES)]
    )


def kernel(x, weight, bias):
    return _run(x, weight, bias, reps=1)


# revision 17
# speedup vs baseline: 2.2664x; 1.1813x over previous
"""Trainium2 Bass kernel v3 for upsample_conv_2d — polyphase conv + DVE FIR.

The baseline folded the 4x4 FIR into the conv weights, turning the op into
four phase-specific 3x3 convs (36 dense 256x256 taps, ~590K PE columns).
This version computes the mathematically minimal dense work instead:

  1. Stage A (PE): the stride-2 3x3 conv_transpose as 9 polyphase taps
     (~150K PE columns, 3.9x less tensor work). PSUM accumulates fp32;
     the Scalar engine evicts each 4-bank PSUM group to an SBUF fp16
     buffer z[131,131] with the two output phases interleaved (strided
     writes) and a zero ring for the FIR padding.
  2. Stage B (DVE + Pool): the separable FIR [1,3,3,1]/2 per dim
     decomposed as three 2-tap box filters per dim
     ([1,3,3,1] = [1,1]*[1,1]*[1,1]), run as tensor_add ops (fp16
     packed => 2x DVE perf mode), each pass row-split ~79/21 between
     the Vector and GpSimd engines. The conv bias is pre-divided by 64
     (the unnormalized FIR mass) and added during eviction + preset on
     the zero ring, so the box chain amplifies it back to exactly +bias.
     The 1/16 FIR normalization is folded into the host-side x and w
     scaling (0.25 each).

Output is staged packed [128, 128*128] fp16 and shipped in 2 DMAs of
2 MB per co-block (16 KB contiguous per partition).
"""

import json

import numpy as np

import concourse.bass as bass
import concourse.mybir as mybir
import concourse.tile as tile
from concourse.bass_utils import run_bass_kernel_spmd

# ---------------------------------------------------------------------------
# BIR post-pass: this walrus build rejects instructions carrying more than one
# sem wait (e.g. Tile's kernel-tail Drain gets 3). Hoist extras into
# standalone EventSemaphore instructions right before the owner.
# ---------------------------------------------------------------------------
_MAX_WAITS = 1


def _split_waits(j: dict) -> dict:
    for fn in j.get("functions", []):
        for blk in fn.get("blocks", []):
            insts = blk.get("instructions")
            if not insts:
                continue
            out = []
            for inst in insts:
                si = inst.get("sync_info") or {}
                waits = si.get("on_wait") or []
                if len(waits) > _MAX_WAITS:
                    for k, w in enumerate(waits[_MAX_WAITS:]):
                        out.append(
                            {
                                "debug": inst.get("debug", 0),
                                "engine": inst["engine"],
                                "ins": [],
                                "name": f"{inst['name']}-wsplit{k}",
                                "opcode": "EventSemaphore",
                                "outs": [],
                                "sync_info": {"on_update": [], "on_wait": [w]},
                            }
                        )
                    si["on_wait"] = waits[:_MAX_WAITS]
                out.append(inst)
            blk["instructions"] = out
    return j


_orig_to_json_bytes = bass.Bass.to_json_bytes


def _patched_to_json_bytes(self):
    return json.dumps(_split_waits(json.loads(_orig_to_json_bytes(self)))).encode()


bass.Bass.to_json_bytes = _patched_to_json_bytes

# ---------------------------------------------------------------------------
# Problem constants (hardcoded; kernel.py must be self-contained)
# ---------------------------------------------------------------------------
N, C, H, W = 8, 256, 64, 64
OH, OW = 2 * H, 2 * W
N_CORES = 8
F32 = mybir.dt.float32
F16 = mybir.dt.float16

# Fraction of each FIR pass's rows run on DVE (rest on GpSimd/Pool).
# 0.62 balances the cost-model rates (DVE 2x fp16 vs Pool 1x at 1.2GHz).
import os as _os

FIR_DVE_FRAC = float(_os.environ.get("FIR_DVE_FRAC", "0.62"))

# Polyphase taps of the stride-2 conv_transpose with full padding:
#   y[2I+pv, 2J+ph] = sum_taps w[p, q] * xpad[I + dr, J + dc]
# where xpad has a 1-px zero halo (xpad[1+i, 1+j] = x[i, j]).
# Entries: (pv, ph, dr, dc, p, q)
TAPS = [
    (0, 0, 0, 0, 0, 0),
    (0, 0, 0, 1, 0, 2),
    (0, 0, 1, 0, 2, 0),
    (0, 0, 1, 1, 2, 2),
    (0, 1, 0, 1, 0, 1),
    (0, 1, 1, 1, 2, 1),
    (1, 0, 1, 0, 1, 0),
    (1, 0, 1, 1, 1, 2),
    (1, 1, 1, 1, 1, 1),
]

# y row/col counts per phase: even phase has 65 outputs, odd has 64.
PHASE_DIMS = {0: 65, 1: 64}


def _tap_weight_matrix(w: np.ndarray) -> np.ndarray:
    """[256,256,3,3] conv_transpose weight -> [128, 36*128] fp16 lhsT matrix.

    Column block index = (tap*2 + cib)*2 + cob; row = ci within ci-block.
    Scaled by 0.25 (with x also scaled 0.25 => 1/16 FIR normalization).
    """
    ws = w.astype(np.float64) * 0.25
    Wmat = np.zeros((128, 36, 128), dtype=np.float16)
    for t, (_pv, _ph, _dr, _dc, p, q) in enumerate(TAPS):
        for cib in range(2):
            for cob in range(2):
                blk = ws[
                    cob * 128 : (cob + 1) * 128,
                    cib * 128 : (cib + 1) * 128,
                    p,
                    q,
                ]  # [co, ci]
                Wmat[:, (t * 2 + cib) * 2 + cob, :] = blk.T.astype(np.float16)
    return Wmat.reshape(128, -1)


def build_nc(reps: int = 1, loop: bool = False) -> bass.Bass:
    nc = bass.Bass("TRN2", target_bir_lowering=False, debug=False)
    x_d = nc.dram_tensor("x", [C, H + 2, W + 2], F16, kind="ExternalInput").ap()
    w_d = nc.dram_tensor("w", [128, 36 * 128], F16, kind="ExternalInput").ap()
    b_d = nc.dram_tensor("bias", [2, 128], F32, kind="ExternalInput").ap()
    out_d = nc.dram_tensor("out", [C, OH, OW], F16, kind="ExternalOutput").ap()

    xb = x_d.rearrange("(b p) h w -> b p h w", p=128)

    IDENT = mybir.ActivationFunctionType.Identity

    with tile.TileContext(nc) as tc:
        with (
            tc.tile_pool(name="const", bufs=1) as cpool,
            tc.tile_pool(name="zbuf", bufs=1) as zpool,
            tc.tile_pool(name="psum", bufs=4, space="PSUM") as ppool,
        ):
            wt = cpool.tile([128, 36, 128], F16)
            nc.sync.dma_start(wt[:], w_d.rearrange("p (a b) -> p a b", b=128))
            bt = cpool.tile([128, 2], F32)
            nc.sync.dma_start(bt[:], b_d.rearrange("b p -> p b"))

            # x arrives zero-padded to 66x66 (and pre-scaled 0.25) from host
            xpad = [
                cpool.tile([128, 66, 66], F16, tag=f"xp{i}", name=f"xp{i}")
                for i in range(2)
            ]
            for cib in range(2):
                nc.sync.dma_start(xpad[cib][:], xb[cib])

            # z: conv_transpose output (y/16) with a 1-px bias/64 ring,
            # phases interleaved. t: FIR ping-pong partner. One (z, t) pair
            # per co-block so the two FIR chains share nothing and overlap
            # freely across engines.
            zb = [
                zpool.tile([128, 131, 131], F16, tag=f"z{i}", name=f"z{i}")
                for i in range(2)
            ]
            tb = [
                zpool.tile([128, 131, 131], F16, tag=f"t{i}", name=f"t{i}")
                for i in range(2)
            ]
            for i in range(2):
                nc.vector.memset(zb[i][:], 0.0)

            def body():
                for cob in range(2):
                    z = zb[cob]
                    t = tb[cob]
                    bcol = bt[:, cob : cob + 1]
                    zv = z[:, 1:131, 1:131].rearrange(
                        "p (i a) (j b) -> p i a j b", a=2, b=2
                    )
                    # Re-init the bias/64 ring (the FIR chain below reuses z
                    # as scratch and clobbers it). The box chain multiplies
                    # the ring + eviction bias by the FIR mass (64),
                    # recovering exactly +bias at every output.
                    for ring in (
                        z[:, 0:1, 0:131],
                        z[:, 130:131, 0:131],
                        z[:, 1:130, 0:1],
                        z[:, 1:130, 130:131],
                    ):
                        nc.scalar.activation(ring, ring, IDENT, bias=bcol, scale=0.0)
                    # ---- Stage A: polyphase matmuls + PSUM eviction ----
                    for pv in (0, 1):
                        for ph in (0, 1):
                            accums = [
                                (dr, dc, cib, (t * 2 + cib) * 2 + cob)
                                for t, (tpv, tph, dr, dc, _p, _q) in enumerate(TAPS)
                                if tpv == pv and tph == ph
                                for cib in range(2)
                            ]
                            rows = PHASE_DIMS[pv]
                            cols = PHASE_DIMS[ph]
                            R = 7 if cols == 65 else 8
                            na = len(accums)
                            i0 = 0
                            while i0 < rows:
                                g_rows = min(2 * R, rows - i0)
                                chunks = []
                                r0 = i0
                                while r0 < i0 + g_rows:
                                    cr = min(R, i0 + g_rows - r0)
                                    chunks.append((r0, cr))
                                    r0 += cr
                                P = ppool.tile([128, 2, 512], F32, tag="mm", name="mm")
                                for ai, (dr, dc, cib, widx) in enumerate(accums):
                                    lhsT = wt[:, widx, :]
                                    for ci_, (cr0, crn) in enumerate(chunks):
                                        rhs = xpad[cib][
                                            :, cr0 + dr : cr0 + dr + crn, dc : dc + cols
                                        ]
                                        outp = P[:, ci_, 0 : crn * cols].rearrange(
                                            "p (r w) -> p r w", w=cols
                                        )
                                        nc.tensor.matmul(
                                            outp,
                                            lhsT,
                                            rhs,
                                            start=(ai == 0),
                                            stop=(ai == na - 1),
                                        )
                                # evict: leading full-R chunks in one op
                                nfull = 0
                                for _cr0, crn in chunks:
                                    if crn == R:
                                        nfull += 1
                                    else:
                                        break
                                bcol = bt[:, cob : cob + 1]
                                if nfull:
                                    src = P[:, 0:nfull, 0 : R * cols].rearrange(
                                        "p c (r w) -> p c r w", w=cols
                                    )
                                    dst = zv[
                                        :, i0 : i0 + nfull * R, pv, 0:cols, ph
                                    ].rearrange("p (c r) j -> p c r j", r=R)
                                    nc.scalar.activation(dst, src, IDENT, bias=bcol)
                                for ci_ in range(nfull, len(chunks)):
                                    cr0, crn = chunks[ci_]
                                    src = P[:, ci_, 0 : crn * cols].rearrange(
                                        "p (r w) -> p r w", w=cols
                                    )
                                    dst = zv[:, cr0 : cr0 + crn, pv, 0:cols, ph]
                                    nc.scalar.activation(dst, src, IDENT, bias=bcol)
                                i0 += g_rows

                    # ---- Stage B: separable FIR as 6 box passes, each
                    # row-split between DVE (fast, 2x fp16) and Pool,
                    # ping-ponging z <-> t ----
                    def split(n):
                        nd = int(round(n * FIR_DVE_FRAC))
                        return ((nc.vector, 0, nd), (nc.gpsimd, nd, n))

                    zf = z[:].rearrange("p a b -> p (a b)")
                    ov = zf[:, 0:16384].rearrange("p (h w) -> p h w", w=128)
                    # (out, in, out_rows, vshift?, cols_out)
                    chain = [
                        (t, z, 130, True, 131),
                        (z, t, 129, True, 131),
                        (t, z, 128, True, 131),
                        (z, t, 128, False, 130),
                        (t, z, 128, False, 129),
                        (ov, t, 128, False, 128),
                    ]
                    for dst, srct, nrows, vert, co_ in chain:
                        for eng, r0, r1 in split(nrows):
                            if r0 == r1:
                                continue
                            if dst is ov:
                                d = ov[:, r0:r1, :]
                            else:
                                d = dst[:, r0:r1, 0:co_]
                            if vert:
                                a = srct[:, r0:r1, 0:co_]
                                b_ = srct[:, r0 + 1 : r1 + 1, 0:co_]
                            else:
                                a = srct[:, r0:r1, 0:co_]
                                b_ = srct[:, r0:r1, 1 : co_ + 1]
                            eng.tensor_add(d, a, b_)
                    for half in range(2):
                        dst = out_d[
                            cob * 128 : (cob + 1) * 128,
                            half * 64 : (half + 1) * 64,
                            :,
                        ].rearrange("c h w -> c (h w)")
                        nc.sync.dma_start(
                            dst, zf[:, half * 8192 : (half + 1) * 8192]
                        )

            if loop:
                with tc.For_i(0, reps):
                    body()
            else:
                for _rep in range(reps):
                    body()
    return nc


_CACHED_NC = {}


def _get_nc(reps: int = 1, loop: bool = False) -> bass.Bass:
    key = (reps, loop)
    if key not in _CACHED_NC:
        _CACHED_NC[key] = build_nc(reps, loop)
    return _CACHED_NC[key]


def _run(x, weight, bias, reps: int = 1, loop: bool = False):
    Wmat = _tap_weight_matrix(np.asarray(weight, dtype=np.float32))
    # bias/64: the box chain multiplies the ring+eviction bias by the
    # unnormalized FIR mass (64), recovering exactly +bias at the output.
    b2 = np.ascontiguousarray(
        (np.asarray(bias, dtype=np.float32) / 64.0).reshape(2, 128)
    )
    xs = (
        np.pad(np.asarray(x, dtype=np.float32), ((0, 0), (0, 0), (1, 1), (1, 1)))
        * 0.25
    ).astype(np.float16)
    nc = _get_nc(reps, loop)
    in_maps = [{"x": xs[i], "w": Wmat, "bias": b2} for i in range(N_CORES)]
    res = run_bass_kernel_spmd(nc, in_maps, list(range(N_CORES)))
    return np.stack(
        [res.results[i]["out"].astype(np.float32) for i in range(N_CORES)]
    )


def kernel(x, weight, bias):
    return _run(x, weight, bias, reps=1)


# revision 19
# speedup vs baseline: 2.9640x; 1.3078x over previous
"""Trainium2 Bass kernel v4 for upsample_conv_2d — polyphase conv + split FIR.

The baseline folded the whole 4x4 FIR into the conv weights, turning the op
into four phase-specific 3x3 convs (36 dense 256x256 taps, ~590K PE columns).
This version balances the work across engines:

  1. Stage A (PE): the stride-2 3x3 conv_transpose composed with ONE
     horizontal 2-tap box filter ([1,1] of the FIR factorization
     [1,3,3,1] = [1,1]*[1,2,1]), as 12 polyphase taps (~202K PE columns,
     2.9x less tensor work than baseline). The box folds into the
     weights (heff = [w0, w0+w1, w1+w2, w2] polyphase), not into extra
     matmul streams. PSUM accumulates fp32; the Scalar engine evicts
     each PSUM group into an SBUF fp16 buffer z[131,130] with the two
     column phases interleaved (strided writes), adding bias/32.
  2. Stage B (DVE + Pool): the remaining FIR — vertical [1,3,3,1]
     (three 2-tap box passes) and horizontal [1,2,1] (two box passes) —
     as tensor_add ops (fp16 packed => 2x DVE perf mode), each pass
     row-split between the Vector and GpSimd engines (GpSimd runs
     tensor ops ~3x slower, so it gets ~24% of rows). Each co-block has
     its own (z, t) buffer pair, ping-ponged, so the two chains share
     nothing and overlap stage A of the other block freely.

The conv bias is pre-divided by 32 (the unnormalized mass of the
remaining FIR chain), added during eviction and preset on the top/bottom
ring rows, so the box chain amplifies it back to exactly +bias. The 1/16
FIR normalization is folded into host-side x and w scaling (0.25 each).
Output is staged packed [128, 128*128] fp16 and shipped in 2 DMAs of
2 MB per co-block (16 KB contiguous per partition).
"""

import json
import os as _os

import numpy as np

import concourse.bass as bass
import concourse.mybir as mybir
import concourse.tile as tile
from concourse.bass_utils import run_bass_kernel_spmd

# ---------------------------------------------------------------------------
# BIR post-pass: this walrus build rejects instructions carrying more than one
# sem wait (e.g. Tile's kernel-tail Drain gets 3). Hoist extras into
# standalone EventSemaphore instructions right before the owner.
# ---------------------------------------------------------------------------
_MAX_WAITS = 1


def _split_waits(j: dict) -> dict:
    for fn in j.get("functions", []):
        for blk in fn.get("blocks", []):
            insts = blk.get("instructions")
            if not insts:
                continue
            out = []
            for inst in insts:
                si = inst.get("sync_info") or {}
                waits = si.get("on_wait") or []
                if len(waits) > _MAX_WAITS:
                    for k, w in enumerate(waits[_MAX_WAITS:]):
                        out.append(
                            {
                                "debug": inst.get("debug", 0),
                                "engine": inst["engine"],
                                "ins": [],
                                "name": f"{inst['name']}-wsplit{k}",
                                "opcode": "EventSemaphore",
                                "outs": [],
                                "sync_info": {"on_update": [], "on_wait": [w]},
                            }
                        )
                    si["on_wait"] = waits[:_MAX_WAITS]
                out.append(inst)
            blk["instructions"] = out
    return j


_orig_to_json_bytes = bass.Bass.to_json_bytes


def _patched_to_json_bytes(self):
    return json.dumps(_split_waits(json.loads(_orig_to_json_bytes(self)))).encode()


bass.Bass.to_json_bytes = _patched_to_json_bytes

# ---------------------------------------------------------------------------
# Problem constants (hardcoded; kernel.py must be self-contained)
# ---------------------------------------------------------------------------
N, C, H, W = 8, 256, 64, 64
OH, OW = 2 * H, 2 * W
N_CORES = 8
F32 = mybir.dt.float32
F16 = mybir.dt.float16

# Fraction of each FIR pass's rows run on DVE (rest on GpSimd/Pool).
# Measured on HW: DVE ~0.61 ns/elem (2x fp16), Pool ~1.89 ns/elem
# (GPSIMD Add runs at ~0.42 of its 1.2GHz roofline).
FIR_DVE_FRAC = float(_os.environ.get("FIR_DVE_FRAC", "0.79"))

# Polyphase taps of (stride-2 conv_transpose, full padding) composed with a
# horizontal 2-tap box [1,1]:
#   z2[1 + 2I + pv, 2J + pc] = sum_taps wsum * xpad[I + dr, J + dc]
# where xpad has a 1-px zero halo and wsum sums the listed (p, q) entries of
# the 3x3 weight. Entries: (pv, pc, dr, dc, [(p, q), ...])
TAPS = []
for pv, vparts in ((0, ((0, 0), (1, 2))), (1, ((1, 1),))):
    # vparts: (dr, p) pairs for this vertical phase
    for pc in (0, 1):
        for dr, p in vparts:
            for dc in (0, 1):
                if pc == 0:
                    qs = [(p, 0), (p, 1)] if dc == 0 else [(p, 2)]
                else:
                    qs = [(p, 0)] if dc == 0 else [(p, 1), (p, 2)]
                TAPS.append((pv, pc, dr, dc, qs))

NT = len(TAPS)  # 12
PHASE_ROWS = {0: 65, 1: 64}


def _tap_weight_matrix(w: np.ndarray) -> np.ndarray:
    """[256,256,3,3] conv_transpose weight -> [128, NT*4*128] fp16 lhsT.

    Column block index = (t*2 + cib)*2 + cob; row = ci within ci-block.
    Scaled 0.25 (with x also scaled 0.25 => the 1/16 FIR normalization).
    """
    ws = w.astype(np.float64) * 0.25
    Wmat = np.zeros((128, NT * 4, 128), dtype=np.float16)
    for t, (_pv, _pc, _dr, _dc, qs) in enumerate(TAPS):
        eff = np.zeros((256, 256), dtype=np.float64)
        for p, q in qs:
            eff += ws[:, :, p, q]
        for cib in range(2):
            for cob in range(2):
                blk = eff[
                    cob * 128 : (cob + 1) * 128, cib * 128 : (cib + 1) * 128
                ]  # [co, ci]
                Wmat[:, (t * 2 + cib) * 2 + cob, :] = blk.T.astype(np.float16)
    return Wmat.reshape(128, -1)


def _prep_inputs(x, weight, bias):
    Wmat = _tap_weight_matrix(np.asarray(weight, dtype=np.float32))
    # bias/32: the remaining FIR chain mass (vertical 8 x horizontal 4)
    # multiplies the ring + eviction bias back to exactly +bias.
    b2 = np.ascontiguousarray(
        (np.asarray(bias, dtype=np.float32) / 32.0).reshape(2, 128)
    )
    xs = (
        np.pad(np.asarray(x, dtype=np.float32), ((0, 0), (0, 0), (1, 1), (1, 1)))
        * 0.25
    ).astype(np.float16)
    return xs, Wmat, b2


def build_nc(reps: int = 1, loop: bool = False) -> bass.Bass:
    nc = bass.Bass("TRN2", target_bir_lowering=False, debug=False)
    x_d = nc.dram_tensor("x", [C, H + 2, W + 2], F16, kind="ExternalInput").ap()
    w_d = nc.dram_tensor("w", [128, NT * 4 * 128], F16, kind="ExternalInput").ap()
    b_d = nc.dram_tensor("bias", [2, 128], F32, kind="ExternalInput").ap()
    out_d = nc.dram_tensor("out", [C, OH, OW], F16, kind="ExternalOutput").ap()

    xb = x_d.rearrange("(b p) h w -> b p h w", p=128)

    IDENT = mybir.ActivationFunctionType.Identity

    with tile.TileContext(nc) as tc:
        with (
            tc.tile_pool(name="const", bufs=1) as cpool,
            tc.tile_pool(name="zbuf", bufs=1) as zpool,
            tc.tile_pool(name="psum", bufs=4, space="PSUM") as ppool,
        ):
            wt = cpool.tile([128, NT * 4, 128], F16)
            nc.sync.dma_start(wt[:], w_d.rearrange("p (a b) -> p a b", b=128))
            bt = cpool.tile([128, 2], F32)
            nc.sync.dma_start(bt[:], b_d.rearrange("b p -> p b"))

            # x arrives zero-padded to 66x66 (and pre-scaled 0.25) from host
            xpad = [
                cpool.tile([128, 66, 66], F16, tag=f"xp{i}", name=f"xp{i}")
                for i in range(2)
            ]
            for cib in range(2):
                nc.sync.dma_start(xpad[cib][:], xb[cib])

            # z: stage-A output (131 rows x 130 cols: bias/32 ring rows 0 and
            # 130, interior rows 1..129 evicted per rep). t: FIR ping-pong
            # partner. One (z, t) pair per co-block so the two FIR chains
            # share nothing.
            zb = [
                zpool.tile([128, 131, 130], F16, tag=f"z{i}", name=f"z{i}")
                for i in range(2)
            ]
            tb = [
                zpool.tile([128, 131, 130], F16, tag=f"t{i}", name=f"t{i}")
                for i in range(2)
            ]
            for i in range(2):
                nc.vector.memset(zb[i][:], 0.0)

            def body():
                for cob in range(2):
                    z = zb[cob]
                    t = tb[cob]
                    bcol = bt[:, cob : cob + 1]
                    zv = z[:, 1:131, :].rearrange(
                        "p (i a) (j b) -> p i a j b", a=2, b=2
                    )
                    # Re-init the bias/32 ring rows (the FIR chain reuses z
                    # as scratch and clobbers row 0; row 130 kept for
                    # symmetry/robustness).
                    for ring in (z[:, 0:1, :], z[:, 130:131, :]):
                        nc.scalar.activation(ring, ring, IDENT, bias=bcol, scale=0.0)
                    # ---- Stage A: polyphase matmuls + PSUM eviction ----
                    for pv in (0, 1):
                        for pc in (0, 1):
                            accums = [
                                (dr, dc, cib, (ti * 2 + cib) * 2 + cob)
                                for ti, (tpv, tpc, dr, dc, _qs) in enumerate(TAPS)
                                if tpv == pv and tpc == pc
                                for cib in range(2)
                            ]
                            rows = PHASE_ROWS[pv]
                            na = len(accums)
                            R = 7
                            i0 = 0
                            while i0 < rows:
                                g_rows = min(2 * R, rows - i0)
                                chunks = []
                                r0 = i0
                                while r0 < i0 + g_rows:
                                    cr = min(R, i0 + g_rows - r0)
                                    chunks.append((r0, cr))
                                    r0 += cr
                                P = ppool.tile([128, 2, 512], F32, tag="mm", name="mm")
                                for ai, (dr, dc, cib, widx) in enumerate(accums):
                                    lhsT = wt[:, widx, :]
                                    for ci_, (cr0, crn) in enumerate(chunks):
                                        rhs = xpad[cib][
                                            :, cr0 + dr : cr0 + dr + crn, dc : dc + 65
                                        ]
                                        outp = P[:, ci_, 0 : crn * 65].rearrange(
                                            "p (r w) -> p r w", w=65
                                        )
                                        nc.tensor.matmul(
                                            outp,
                                            lhsT,
                                            rhs,
                                            start=(ai == 0),
                                            stop=(ai == na - 1),
                                        )
                                # evict: uniform leading chunks in one op
                                nfull = 0
                                for _cr0, crn in chunks:
                                    if crn == R:
                                        nfull += 1
                                    else:
                                        break
                                if nfull:
                                    src = P[:, 0:nfull, 0 : R * 65].rearrange(
                                        "p c (r w) -> p c r w", w=65
                                    )
                                    dst = zv[
                                        :, i0 : i0 + nfull * R, pv, 0:65, pc
                                    ].rearrange("p (c r) j -> p c r j", r=R)
                                    nc.scalar.activation(dst, src, IDENT, bias=bcol)
                                for ci_ in range(nfull, len(chunks)):
                                    cr0, crn = chunks[ci_]
                                    src = P[:, ci_, 0 : crn * 65].rearrange(
                                        "p (r w) -> p r w", w=65
                                    )
                                    dst = zv[:, cr0 : cr0 + crn, pv, 0:65, pc]
                                    nc.scalar.activation(dst, src, IDENT, bias=bcol)
                                i0 += g_rows

                    # ---- Stage B: remaining FIR (vertical [1,3,3,1] +
                    # horizontal [1,2,1]) as 5 box passes, each row-split
                    # between DVE and Pool, ping-ponging z <-> t ----
                    def split(n):
                        nd = int(round(n * FIR_DVE_FRAC))
                        return ((nc.vector, 0, nd), (nc.gpsimd, nd, n))

                    tf = t[:].rearrange("p a b -> p (a b)")
                    ov = tf[:, 0:16384].rearrange("p (h w) -> p h w", w=128)
                    # (out, in, out_rows, vshift?, cols_out)
                    chain = [
                        (t, z, 130, True, 130),
                        (z, t, 129, True, 130),
                        (t, z, 128, True, 130),
                        (z, t, 128, False, 129),
                        (ov, z, 128, False, 128),
                    ]
                    for dst, srct, nrows, vert, co_ in chain:
                        for eng, r0, r1 in split(nrows):
                            if r0 == r1:
                                continue
                            if dst is ov:
                                d = ov[:, r0:r1, :]
                            else:
                                d = dst[:, r0:r1, 0:co_]
                            if vert:
                                a = srct[:, r0:r1, 0:co_]
                                b_ = srct[:, r0 + 1 : r1 + 1, 0:co_]
                            else:
                                a = srct[:, r0:r1, 0:co_]
                                b_ = srct[:, r0:r1, 1 : co_ + 1]
                            eng.tensor_add(d, a, b_)
                    for half in range(2):
                        dst = out_d[
                            cob * 128 : (cob + 1) * 128,
                            half * 64 : (half + 1) * 64,
                            :,
                        ].rearrange("c h w -> c (h w)")
                        nc.sync.dma_start(dst, tf[:, half * 8192 : (half + 1) * 8192])

            if loop:
                with tc.For_i(0, reps):
                    body()
            else:
                for _rep in range(reps):
                    body()
    return nc


_CACHED_NC = {}


def _get_nc(reps: int = 1, loop: bool = False) -> bass.Bass:
    key = (reps, loop)
    if key not in _CACHED_NC:
        _CACHED_NC[key] = build_nc(reps, loop)
    return _CACHED_NC[key]


def _run(x, weight, bias, reps: int = 1, loop: bool = False):
    xs, Wmat, b2 = _prep_inputs(x, weight, bias)
    nc = _get_nc(reps, loop)
    in_maps = [{"x": xs[i], "w": Wmat, "bias": b2} for i in range(N_CORES)]
    res = run_bass_kernel_spmd(nc, in_maps, list(range(N_CORES)))
    return np.stack(
        [res.results[i]["out"].astype(np.float32) for i in range(N_CORES)]
    )


def kernel(x, weight, bias):
    return _run(x, weight, bias, reps=1)
